# revision 2
# baseline (speedup 1.0000x reference)
"""Trainium2 Bass kernel for nn_ClusteringLayer (VQ codebook assign + gather).

Per-core pipeline (data-parallel over batch, 16384 tokens/core, 128 tiles):
  - PE: 3 fp16 matmuls per 128-token tile -> PSUM fp32 biased scores
    (bias row adds -0.5||c_k||^2 + 128 via bf16 hi/mid/lo rows).
  - DVE: ONE custom op (ENC_MAX_ANT) per tile encodes
      enc_k = round_{2^-5}(score_k) * 2^5 + k*2^-10 + 8192
    and max-reduces it: the accumulator holds the quantized max score AND
    the argmax index in one 512-element pass (vs Max8+MaxIndex = two).
  - Slab-batched tiny DVE ops decode the index (u32) and the ACT bias.
  - ACT: relu(score + (delta - m)) with accum_out = margin mass; tokens with
    competitors within ~0.1 of the max get flagged for exact host rescore.
  - GPSIMD indirect DMA gathers centers[idx] rows; y written per slab.
Host: exact fp32 rescore of flagged tokens (~2-3%) => exact argmax overall.
"""
from contextlib import ExitStack

import numpy as np
import ml_dtypes

import concourse.bass as bass
import concourse.bacc as bacc
import concourse.mybir as mybir
import concourse.tile as tile
import concourse.bass_utils as bass_utils
import concourse.dve_ops as dve_ops
from concourse.dve_spec import Spec, Src0, Src1, C0, C1, C2, maxx, lower
from concourse.dve_uop import DveOpSpec
from concourse.dve_ops import DveOp

B, H, W, C = 32, 64, 64, 256
K = 512
N_CORES = 8
P = 128
NTOK = B * H * W // N_CORES
NTILES = NTOK // P
SLAB = 4
NSLAB = NTILES // SLAB

F32 = mybir.dt.float32
F16 = mybir.dt.float16
BF = mybir.dt.bfloat16
U32 = mybir.dt.uint32

MAGIC = 1.5 * 2**31     # fp32 magic: add rounds x*2^13 to multiples of 2^8
SCALE_IN = float(2**13)
SCALE_OUT = float(2**-8)
M3 = 1.5 * 2**23        # rounds enc to integer
DELTA = 0.15
QSTEP = 2.0**-5
T_FLAG = DELTA + QSTEP / 2 + 0.01

_NC_CACHE = {}

ENC_SPEC = Spec(
    body=((Src0 * C0 + C1) - C1) * C2 + Src1,
    accum=maxx,
    reference=None,
)


def _register_encmax():
    name = "ENC_MAX_ANT"
    for o in dve_ops.OPS:
        if o.name == name:
            return o
    row = max(dve_ops._SUB_OPCODE_FOR_NAME.values()) + 1
    assert row < 0x20
    dve_ops._SUB_OPCODE_FOR_NAME[name] = row
    shas = {}
    for ver in ("v3", "v4"):
        try:
            sl = DveOpSpec(name=name, opcode=row,
                           uops=lower(ENC_SPEC, ver=ver), rd1_en=True)
            shas[ver] = sl.sha(ver)
        except Exception:
            pass
    op = DveOp(name, ENC_SPEC, subdim=False, uops_sha=shas)
    dve_ops.OPS.append(op)
    dve_ops.CUSTOM_DVE_SPECS[name] = ENC_SPEC
    return op


ENC_MAX = _register_encmax()


def _build(ntok: int, num_devices: int):
    ntiles = ntok // P
    nslab = ntiles // SLAB

    nc = bacc.Bacc("TRN2", target_bir_lowering=False, debug=False,
                   num_devices=num_devices)
    xT_d = nc.dram_tensor("xT", [C, ntok], F16, kind="ExternalInput").ap()
    cT_d = nc.dram_tensor("cT", [C, K], F16, kind="ExternalInput").ap()
    b3_d = nc.dram_tensor("bias3", [3, K], BF, kind="ExternalInput").ap()
    o3_d = nc.dram_tensor("ones3", [3, P], BF, kind="ExternalInput").ap()
    iot_d = nc.dram_tensor("iota", [P, K], F32, kind="ExternalInput").ap()
    cent_d = nc.dram_tensor("centers", [K, C], F32, kind="ExternalInput").ap()

    y_d = nc.dram_tensor("y", [ntok, C], F32, kind="ExternalOutput").ap()
    enc_d = nc.dram_tensor("encm", [P, ntiles], F32, kind="ExternalOutput").ap()
    sr_d = nc.dram_tensor("sumrelu", [P, ntiles], F32, kind="ExternalOutput").ap()

    xT_v = xT_d.rearrange("(h p) n -> p h n", h=2)
    y_slab = y_d.rearrange("(t p) c -> p t c", p=P)

    with tile.TileContext(nc) as tc, ExitStack() as ctx:
        constp = ctx.enter_context(tc.tile_pool(name="const", bufs=1))
        xp = ctx.enter_context(tc.tile_pool(name="x", bufs=4))
        scp = ctx.enter_context(tc.tile_pool(name="scratch", bufs=3))
        rlp = ctx.enter_context(tc.tile_pool(name="rl", bufs=3))
        yp = ctx.enter_context(tc.tile_pool(name="y", bufs=4))
        accp = ctx.enter_context(tc.tile_pool(name="acc", bufs=1))
        psump = ctx.enter_context(tc.tile_pool(name="psum", bufs=8, space="PSUM"))

        cT = constp.tile([P, 2, K], F16, tag="cT")
        nc.sync.dma_start(cT[:], cT_d.rearrange("(h p) n -> p h n", h=2))
        b3 = constp.tile([3, K], BF, tag="b3")
        nc.sync.dma_start(b3[:], b3_d[:])
        o3 = constp.tile([3, P], BF, tag="o3")
        nc.sync.dma_start(o3[:], o3_d[:])
        iot = constp.tile([P, K], F32, tag="iota")
        nc.sync.dma_start(iot[:], iot_d[:])

        enc_all = accp.tile([P, ntiles], F32, tag="enc_all")
        sr_all = accp.tile([P, ntiles], F32, tag="sr_all")
        ku_all = accp.tile([P, ntiles], U32, tag="ku_all")
        r_all = accp.tile([P, ntiles], F32, tag="r_all")
        kf_all = accp.tile([P, ntiles], F32, tag="kf_all")
        mb_all = accp.tile([P, ntiles], F32, tag="mb_all")

        SL = SLAB * P
        for s in range(nslab):
            t0 = s * SLAB
            xs = xp.tile([P, 2, SL], F16, tag="xs")
            nc.sync.dma_start(xs[:], xT_v[:, :, bass.ts(s, SL)])

            pss = []
            for j in range(SLAB):
                t = t0 + j
                ps = psump.tile([P, K], F32, tag="ps")
                pss.append(ps)
                nc.tensor.matmul(ps[:], o3[:], b3[:], start=True, stop=False)
                nc.tensor.matmul(ps[:], xs[:, 0, bass.ts(j, P)], cT[:, 0, :],
                                 start=False, stop=False)
                nc.tensor.matmul(ps[:], xs[:, 1, bass.ts(j, P)], cT[:, 1, :],
                                 start=False, stop=True)
                encs = scp.tile([P, K], BF, tag="encs")
                nc.vector._custom_dve(
                    ENC_MAX, out=encs[:], in0=ps[:], in1=iot[:],
                    s0=SCALE_IN, s1=MAGIC, imm2=SCALE_OUT,
                    accum_out=enc_all[:, t:t + 1])

            sl4 = slice(t0, t0 + SLAB)
            nc.vector.tensor_scalar(r_all[:, sl4], enc_all[:, sl4], M3, -M3,
                                    mybir.AluOpType.add, mybir.AluOpType.add)
            nc.vector.tensor_tensor(kf_all[:, sl4], enc_all[:, sl4],
                                    r_all[:, sl4], op=mybir.AluOpType.subtract)
            nc.vector.tensor_scalar(ku_all[:, sl4], kf_all[:, sl4],
                                    float(2**10), None, mybir.AluOpType.mult)
            nc.vector.tensor_scalar(mb_all[:, sl4], r_all[:, sl4],
                                    float(-(2**-5)), DELTA + 256.0,
                                    mybir.AluOpType.mult, mybir.AluOpType.add)

            yg = yp.tile([P, SLAB, C], F32, tag="yg")
            for j in range(SLAB):
                t = t0 + j
                rl = rlp.tile([P, K], F16, tag="rl")
                nc.scalar.activation(rl[:], pss[j][:],
                                     mybir.ActivationFunctionType.Relu,
                                     bias=mb_all[:, t:t + 1], scale=1.0,
                                     accum_out=sr_all[:, t:t + 1])
                nc.gpsimd.indirect_dma_start(
                    out=yg[:, j, :], out_offset=None, in_=cent_d[:],
                    in_offset=bass.IndirectOffsetOnAxis(
                        ap=ku_all[:, t:t + 1], axis=0))
            nc.sync.dma_start(y_slab[:, bass.ts(s, SLAB), :], yg[:])

        nc.sync.dma_start(enc_d[:], enc_all[:])
        nc.sync.dma_start(sr_d[:], sr_all[:])

    nc.compile()
    return nc


def _shared_inputs(centers: np.ndarray):
    bf16 = ml_dtypes.bfloat16
    cT = np.ascontiguousarray(centers.T.astype(np.float16))
    c_sq = (centers.astype(np.float64) ** 2).sum(-1)
    bias = -0.5 * c_sq + 128.0
    b0 = bias.astype(bf16)
    b1 = (bias - b0.astype(np.float64)).astype(bf16)
    b2 = (bias - b0.astype(np.float64) - b1.astype(np.float64)).astype(bf16)
    bias3 = np.ascontiguousarray(np.stack([b0, b1, b2]).astype(bf16))
    ones3 = np.ones((3, P), dtype=bf16)
    iota = np.ascontiguousarray(np.broadcast_to(
        (np.arange(K, dtype=np.float64) * 2**-10 + 8192.0).astype(np.float32),
        (P, K)))
    return {"cT": cT, "bias3": bias3, "ones3": ones3, "iota": iota,
            "centers": centers}


def kernel(x: np.ndarray, centers: np.ndarray):
    x = np.asarray(x)
    centers = np.ascontiguousarray(np.asarray(centers, dtype=np.float32))
    assert x.shape == (B, H, W, C) and centers.shape == (K, C)

    key = (NTOK, N_CORES)
    if key not in _NC_CACHE:
        _NC_CACHE[key] = _build(NTOK, N_CORES)
    nc = _NC_CACHE[key]

    shared = _shared_inputs(centers)
    flat = np.ascontiguousarray(x, dtype=np.float32).reshape(N_CORES, NTOK, C)
    in_maps = []
    for c in range(N_CORES):
        xT = np.ascontiguousarray(flat[c].T.astype(np.float16))
        in_maps.append({"xT": xT, **shared})

    res = bass_utils.run_bass_kernel_spmd(nc, in_maps,
                                          core_ids=list(range(N_CORES)))

    c_sq32 = (centers * centers).sum(-1)
    y = np.empty((N_CORES, NTOK, C), dtype=np.float32)
    for c in range(N_CORES):
        yc = res.results[c]["y"].copy()
        sr = res.results[c]["sumrelu"]           # [P, ntiles]
        flag_tok = (sr.T.reshape(-1) > T_FLAG)   # token = tile*128 + p
        if flag_tok.any():
            xf = flat[c][flag_tok]
            d = c_sq32[None, :] - 2.0 * (xf @ centers.T)
            yc[flag_tok] = centers[d.argmin(-1)]
        y[c] = yc

    return (x, y.reshape(B, H, W, C))


# revision 3
# speedup vs baseline: 1.0344x; 1.0344x over previous
"""Trainium2 Bass kernel for nn_ClusteringLayer (VQ codebook assign + gather).

Per-core pipeline (data-parallel over batch, 16384 tokens/core, 128 tiles):
  - PE: 3 fp16 matmuls per 128-token tile -> PSUM fp32 biased scores
    (bias row adds -0.5||c_k||^2 + 128 via bf16 hi/mid/lo rows).
  - DVE: ONE custom op (ENC_MAX_ANT) per tile encodes
      enc_k = round_{2^-5}(score_k) * 2^5 + k*2^-10 + 8192
    and max-reduces it: the accumulator holds the quantized max score AND
    the argmax index in one 512-element pass (vs Max8+MaxIndex = two).
  - Slab-batched tiny DVE ops decode the index (u32) and the ACT bias.
  - ACT: relu(score + (delta - m)) with accum_out = margin mass; tokens with
    competitors within ~0.1 of the max get flagged for exact host rescore.
  - GPSIMD indirect DMA gathers centers[idx] rows; y written per slab.
Host: exact fp32 rescore of flagged tokens (~2-3%) => exact argmax overall.
"""
from contextlib import ExitStack

import numpy as np
import ml_dtypes

import concourse.bass as bass
import concourse.bacc as bacc
import concourse.mybir as mybir
import concourse.tile as tile
import concourse.bass_utils as bass_utils
import concourse.dve_ops as dve_ops
from concourse.dve_spec import Spec, Src0, Src1, C0, C1, C2, maxx, lower
from concourse.dve_uop import DveOpSpec
from concourse.dve_ops import DveOp

B, H, W, C = 32, 64, 64, 256
K = 512
N_CORES = 8
P = 128
NTOK = B * H * W // N_CORES
NTILES = NTOK // P
SLAB = 2
NSLAB = NTILES // SLAB

F32 = mybir.dt.float32
F16 = mybir.dt.float16
BF = mybir.dt.bfloat16
U32 = mybir.dt.uint32

MAGIC = 1.5 * 2**31     # fp32 magic: add rounds x*2^13 to multiples of 2^8
SCALE_IN = float(2**13)
SCALE_OUT = float(2**-8)
M3 = 1.5 * 2**23        # rounds enc to integer
DELTA = 0.15
QSTEP = 2.0**-5
T_FLAG = DELTA + QSTEP / 2 + 0.01

_NC_CACHE = {}

ENC_SPEC = Spec(
    body=((Src0 * C0 + C1) - C1) * C2 + Src1,
    accum=maxx,
    reference=None,
)


def _register_encmax():
    name = "ENC_MAX_ANT"
    for o in dve_ops.OPS:
        if o.name == name:
            return o
    row = max(dve_ops._SUB_OPCODE_FOR_NAME.values()) + 1
    assert row < 0x20
    dve_ops._SUB_OPCODE_FOR_NAME[name] = row
    shas = {}
    for ver in ("v3", "v4"):
        try:
            sl = DveOpSpec(name=name, opcode=row,
                           uops=lower(ENC_SPEC, ver=ver), rd1_en=True)
            shas[ver] = sl.sha(ver)
        except Exception:
            pass
    op = DveOp(name, ENC_SPEC, subdim=False, uops_sha=shas)
    dve_ops.OPS.append(op)
    dve_ops.CUSTOM_DVE_SPECS[name] = ENC_SPEC
    return op


ENC_MAX = _register_encmax()


def _build(ntok: int, num_devices: int):
    ntiles = ntok // P
    nslab = ntiles // SLAB

    nc = bacc.Bacc("TRN2", target_bir_lowering=False, debug=False,
                   num_devices=num_devices)
    xT_d = nc.dram_tensor("xT", [C, ntok], F16, kind="ExternalInput").ap()
    cT_d = nc.dram_tensor("cT", [C, K], F16, kind="ExternalInput").ap()
    b3_d = nc.dram_tensor("bias3", [3, K], BF, kind="ExternalInput").ap()
    o3_d = nc.dram_tensor("ones3", [3, P], BF, kind="ExternalInput").ap()
    iot_d = nc.dram_tensor("iota", [P, K], F32, kind="ExternalInput").ap()
    cent_d = nc.dram_tensor("centers", [K, C], F32, kind="ExternalInput").ap()

    y_d = nc.dram_tensor("y", [ntok, C], F32, kind="ExternalOutput").ap()
    enc_d = nc.dram_tensor("encm", [P, ntiles], F32, kind="ExternalOutput").ap()
    sr_d = nc.dram_tensor("sumrelu", [P, ntiles], F32, kind="ExternalOutput").ap()

    xT_v = xT_d.rearrange("(h p) n -> p h n", h=2)
    y_slab = y_d.rearrange("(t p) c -> p t c", p=P)

    with tile.TileContext(nc) as tc, ExitStack() as ctx:
        constp = ctx.enter_context(tc.tile_pool(name="const", bufs=1))
        xp = ctx.enter_context(tc.tile_pool(name="x", bufs=4))
        scp = ctx.enter_context(tc.tile_pool(name="scratch", bufs=3))
        rlp = ctx.enter_context(tc.tile_pool(name="rl", bufs=3))
        yp = ctx.enter_context(tc.tile_pool(name="y", bufs=4))
        accp = ctx.enter_context(tc.tile_pool(name="acc", bufs=1))
        psump = ctx.enter_context(tc.tile_pool(name="psum", bufs=8, space="PSUM"))

        cT = constp.tile([P, 2, K], F16, tag="cT")
        nc.sync.dma_start(cT[:], cT_d.rearrange("(h p) n -> p h n", h=2))
        b3 = constp.tile([3, K], BF, tag="b3")
        nc.sync.dma_start(b3[:], b3_d[:])
        o3 = constp.tile([3, P], BF, tag="o3")
        nc.sync.dma_start(o3[:], o3_d[:])
        iot = constp.tile([P, K], F32, tag="iota")
        nc.sync.dma_start(iot[:], iot_d[:])

        enc_all = accp.tile([P, ntiles], F32, tag="enc_all")
        sr_all = accp.tile([P, ntiles], F32, tag="sr_all")
        ku_all = accp.tile([P, ntiles], U32, tag="ku_all")
        r_all = accp.tile([P, ntiles], F32, tag="r_all")
        kf_all = accp.tile([P, ntiles], F32, tag="kf_all")
        mb_all = accp.tile([P, ntiles], F32, tag="mb_all")

        SL = SLAB * P
        for s in range(nslab):
            t0 = s * SLAB
            xs = xp.tile([P, 2, SL], F16, tag="xs")
            nc.sync.dma_start(xs[:], xT_v[:, :, bass.ts(s, SL)])

            pss = []
            for j in range(SLAB):
                t = t0 + j
                ps = psump.tile([P, K], F32, tag="ps")
                pss.append(ps)
                nc.tensor.matmul(ps[:], o3[:], b3[:], start=True, stop=False)
                nc.tensor.matmul(ps[:], xs[:, 0, bass.ts(j, P)], cT[:, 0, :],
                                 start=False, stop=False)
                nc.tensor.matmul(ps[:], xs[:, 1, bass.ts(j, P)], cT[:, 1, :],
                                 start=False, stop=True)
                encs = scp.tile([P, K], BF, tag="encs")
                nc.vector._custom_dve(
                    ENC_MAX, out=encs[:], in0=ps[:], in1=iot[:],
                    s0=SCALE_IN, s1=MAGIC, imm2=SCALE_OUT,
                    accum_out=enc_all[:, t:t + 1])

            sl4 = slice(t0, t0 + SLAB)
            nc.vector.tensor_scalar(r_all[:, sl4], enc_all[:, sl4], M3, -M3,
                                    mybir.AluOpType.add, mybir.AluOpType.add)
            nc.vector.tensor_tensor(kf_all[:, sl4], enc_all[:, sl4],
                                    r_all[:, sl4], op=mybir.AluOpType.subtract)
            nc.vector.tensor_scalar(ku_all[:, sl4], kf_all[:, sl4],
                                    float(2**10), None, mybir.AluOpType.mult)
            nc.vector.tensor_scalar(mb_all[:, sl4], r_all[:, sl4],
                                    float(-(2**-5)), DELTA + 256.0,
                                    mybir.AluOpType.mult, mybir.AluOpType.add)

            yg = yp.tile([P, SLAB, C], F32, tag="yg")
            for j in range(SLAB):
                t = t0 + j
                rl = rlp.tile([P, K], F16, tag="rl")
                nc.scalar.activation(rl[:], pss[j][:],
                                     mybir.ActivationFunctionType.Relu,
                                     bias=mb_all[:, t:t + 1], scale=1.0,
                                     accum_out=sr_all[:, t:t + 1])
                nc.gpsimd.indirect_dma_start(
                    out=yg[:, j, :], out_offset=None, in_=cent_d[:],
                    in_offset=bass.IndirectOffsetOnAxis(
                        ap=ku_all[:, t:t + 1], axis=0))
            nc.sync.dma_start(y_slab[:, bass.ts(s, SLAB), :], yg[:])

        nc.sync.dma_start(enc_d[:], enc_all[:])
        nc.sync.dma_start(sr_d[:], sr_all[:])

    nc.compile()
    return nc


def _shared_inputs(centers: np.ndarray):
    bf16 = ml_dtypes.bfloat16
    cT = np.ascontiguousarray(centers.T.astype(np.float16))
    c_sq = (centers.astype(np.float64) ** 2).sum(-1)
    bias = -0.5 * c_sq + 128.0
    b0 = bias.astype(bf16)
    b1 = (bias - b0.astype(np.float64)).astype(bf16)
    b2 = (bias - b0.astype(np.float64) - b1.astype(np.float64)).astype(bf16)
    bias3 = np.ascontiguousarray(np.stack([b0, b1, b2]).astype(bf16))
    ones3 = np.ones((3, P), dtype=bf16)
    iota = np.ascontiguousarray(np.broadcast_to(
        (np.arange(K, dtype=np.float64) * 2**-10 + 8192.0).astype(np.float32),
        (P, K)))
    return {"cT": cT, "bias3": bias3, "ones3": ones3, "iota": iota,
            "centers": centers}


def kernel(x: np.ndarray, centers: np.ndarray):
    x = np.asarray(x)
    centers = np.ascontiguousarray(np.asarray(centers, dtype=np.float32))
    assert x.shape == (B, H, W, C) and centers.shape == (K, C)

    key = (NTOK, N_CORES)
    if key not in _NC_CACHE:
        _NC_CACHE[key] = _build(NTOK, N_CORES)
    nc = _NC_CACHE[key]

    shared = _shared_inputs(centers)
    flat = np.ascontiguousarray(x, dtype=np.float32).reshape(N_CORES, NTOK, C)
    in_maps = []
    for c in range(N_CORES):
        xT = np.ascontiguousarray(flat[c].T.astype(np.float16))
        in_maps.append({"xT": xT, **shared})

    res = bass_utils.run_bass_kernel_spmd(nc, in_maps,
                                          core_ids=list(range(N_CORES)))

    c_sq32 = (centers * centers).sum(-1)
    y = np.empty((N_CORES, NTOK, C), dtype=np.float32)
    for c in range(N_CORES):
        yc = res.results[c]["y"].copy()
        sr = res.results[c]["sumrelu"]           # [P, ntiles]
        flag_tok = (sr.T.reshape(-1) > T_FLAG)   # token = tile*128 + p
        if flag_tok.any():
            xf = flat[c][flag_tok]
            d = c_sq32[None, :] - 2.0 * (xf @ centers.T)
            yc[flag_tok] = centers[d.argmin(-1)]
        y[c] = yc

    return (x, y.reshape(B, H, W, C))
